# revision 3
# baseline (speedup 1.0000x reference)
"""Trainium2 Bass kernel for nn_Loss_40080634806416.

The reference module's output is:
  out[:, 0:4] = (bx, by, bw, bh) scalars computed from x[0,0:2], y[0,0:2]
  out[:, 4]   = 0
  out[:, 5:]  = y[:, 5:]

So this is a memory-regime kernel: one big row-sharded copy of y with the
first five columns of every row overwritten by a broadcast header.

Sharding: data-parallel over the box dim N (500000 rows) across 8 cores,
62500 rows per core; the four scalar decode terms are computed on-device
from a tiny replicated header input.
"""

import numpy as np

N = 500000
D = 85
N_CORES = 8
ROWS_PER_CORE = N // N_CORES          # 62500
P = 125                                # SBUF partitions used per tile
RPP = 50                               # rows packed per partition
TILE_ROWS = P * RPP                    # 6250 rows per tile
N_TILES = ROWS_PER_CORE // TILE_ROWS   # 10 tiles per core, no remainder
AC = 1e-16
DATA_BUFS = 4

_CACHE = {}


def build_nc():
    import concourse.bass as bass
    import concourse.tile as tile
    from concourse import bacc, mybir

    FT = mybir.ActivationFunctionType

    # Bacc (not raw Bass): its compile pass splits multi-semaphore waits into
    # event semaphores, which TRN2 compute instructions require (max 1 wait).
    nc = bacc.Bacc()
    y_d = nc.declare_dram_parameter("y", [ROWS_PER_CORE, D], mybir.dt.float32, isOutput=False)
    h_d = nc.declare_dram_parameter("h", [P, 4], mybir.dt.float32, isOutput=False)
    o_d = nc.declare_dram_parameter("o", [ROWS_PER_CORE, D], mybir.dt.float32, isOutput=True)

    with tile.TileContext(nc) as tc:
        with tc.tile_pool(name="const", bufs=1) as cpool, \
             tc.tile_pool(name="data", bufs=DATA_BUFS) as dpool:
            # ---- one-time header computation -------------------------------
            # h rows are all [px, py, tx, ty] (replicated across partitions
            # host-side so no cross-partition broadcast is needed on device).
            hb = cpool.tile([P, 4], mybir.dt.float32)
            nc.sync.dma_start(hb[:, :], h_d[:, :])
            sg = cpool.tile([P, 2], mybir.dt.float32)
            ex = cpool.tile([P, 2], mybir.dt.float32)
            head = cpool.tile([P, RPP * 5], mybir.dt.float32)
            nc.scalar.activation(sg[:, :], hb[:, 0:2], FT.Sigmoid)
            nc.scalar.activation(ex[:, :], hb[:, 0:2], FT.Exp)
            # head[:, 0:2] = sigmoid(px,py) + (tx,ty); head[:, 2:4] = AC*exp(px,py)
            nc.vector.tensor_add(head[:, 0:2], sg[:, :], hb[:, 2:4])
            nc.scalar.mul(head[:, 2:4], ex[:, :], AC)
            nc.vector.memset(head[:, 4:5], 0.0)
            # replicate the 5-wide header RPP times along the free dim
            w = 5
            while w < RPP * 5:
                c = min(w, RPP * 5 - w)
                nc.vector.tensor_copy(head[:, w:w + c], head[:, 0:c])
                w += c

            y_t = y_d.rearrange("(t p r) c -> t p (r c)", p=P, r=RPP)
            o_t = o_d.rearrange("(t p r) c -> t p (r c)", p=P, r=RPP)

            # ---- steady state: load rows, patch header columns, store ------
            for i in range(N_TILES):
                t = dpool.tile([P, RPP * D], mybir.dt.float32)
                nc.sync.dma_start(t[:, :], y_t[i])
                tv = t.rearrange("p (r c) -> p r c", c=D)
                hv = head.rearrange("p (r c) -> p r c", c=5)
                nc.vector.tensor_copy(tv[:, :, 0:5], hv[:, :, :])
                nc.scalar.dma_start(o_t[i], t[:, :])
    nc.finalize()
    return nc


def _get_nc():
    if "nc" not in _CACHE:
        _CACHE["nc"] = build_nc()
    return _CACHE["nc"]


def make_in_maps(x: np.ndarray, y: np.ndarray):
    h = np.empty((P, 4), np.float32)
    h[:, 0] = x[0, 0]
    h[:, 1] = x[0, 1]
    h[:, 2] = y[0, 0]
    h[:, 3] = y[0, 1]
    return [
        {"y": np.ascontiguousarray(y[i * ROWS_PER_CORE:(i + 1) * ROWS_PER_CORE]), "h": h}
        for i in range(N_CORES)
    ]


def kernel(x: np.ndarray, y: np.ndarray) -> np.ndarray:
    from concourse.bass_utils import run_bass_kernel_spmd

    nc = _get_nc()
    in_maps = make_in_maps(x, y)
    res = run_bass_kernel_spmd(nc, in_maps, core_ids=list(range(N_CORES)))
    return np.concatenate([res.results[i]["o"] for i in range(N_CORES)], axis=0)


# revision 4
# speedup vs baseline: 1.5199x; 1.5199x over previous
"""Trainium2 Bass kernel for nn_Loss_40080634806416.

The reference module's output is:
  out[:, 0:4] = (bx, by, bw, bh) scalars computed from x[0,0:2], y[0,0:2]
  out[:, 4]   = 0
  out[:, 5:]  = y[:, 5:]

So this is a memory-regime kernel: one big row-sharded copy of y with the
first five columns of every row overwritten by a broadcast header.

Sharding: data-parallel over the box dim N (500000 rows) across 8 cores,
62500 rows per core; the four scalar decode terms are computed on-device
from a tiny replicated header input.
"""

import numpy as np

N = 500000
D = 85
N_CORES = 8
ROWS_PER_CORE = N // N_CORES          # 62500
P = 125                                # SBUF partitions used per tile
RPP = 50                               # rows packed per partition
TILE_ROWS = P * RPP                    # 6250 rows per tile
N_TILES = ROWS_PER_CORE // TILE_ROWS   # 10 tiles per core, no remainder
AC = 1e-16
DATA_BUFS = 4

_CACHE = {}


def build_nc():
    import concourse.bass as bass
    import concourse.tile as tile
    from concourse import bacc, mybir

    FT = mybir.ActivationFunctionType

    # Bacc (not raw Bass): its compile pass splits multi-semaphore waits into
    # event semaphores, which TRN2 compute instructions require (max 1 wait).
    nc = bacc.Bacc()
    y_d = nc.declare_dram_parameter("y", [ROWS_PER_CORE, D], mybir.dt.float32, isOutput=False)
    h_d = nc.declare_dram_parameter("h", [P, 4], mybir.dt.float32, isOutput=False)
    o_d = nc.declare_dram_parameter("o", [ROWS_PER_CORE, D], mybir.dt.float32, isOutput=True)

    with tile.TileContext(nc) as tc:
        with tc.tile_pool(name="const", bufs=1) as cpool, \
             tc.tile_pool(name="data", bufs=DATA_BUFS) as dpool:
            # ---- one-time header computation -------------------------------
            # h rows are all [px, py, tx, ty] (replicated across partitions
            # host-side so no cross-partition broadcast is needed on device).
            hb = cpool.tile([P, 4], mybir.dt.float32)
            nc.sync.dma_start(hb[:, :], h_d[:, :])
            sg = cpool.tile([P, 2], mybir.dt.float32)
            ex = cpool.tile([P, 2], mybir.dt.float32)
            head = cpool.tile([P, RPP * 5], mybir.dt.float32)
            nc.scalar.activation(sg[:, :], hb[:, 0:2], FT.Sigmoid)
            nc.scalar.activation(ex[:, :], hb[:, 0:2], FT.Exp)
            # head[:, 0:2] = sigmoid(px,py) + (tx,ty); head[:, 2:4] = AC*exp(px,py)
            nc.vector.tensor_add(head[:, 0:2], sg[:, :], hb[:, 2:4])
            nc.scalar.mul(head[:, 2:4], ex[:, :], AC)
            nc.vector.memset(head[:, 4:5], 0.0)
            # replicate the 5-wide header RPP times along the free dim
            w = 5
            while w < RPP * 5:
                c = min(w, RPP * 5 - w)
                nc.vector.tensor_copy(head[:, w:w + c], head[:, 0:c])
                w += c

            y_t = y_d.rearrange("(t p r) c -> t p (r c)", p=P, r=RPP)
            o_t = o_d.rearrange("(t p r) c -> t p (r c)", p=P, r=RPP)

            # ---- steady state: load rows, patch header columns, store ------
            for i in range(N_TILES):
                t = dpool.tile([P, RPP * D], mybir.dt.float32)
                nc.gpsimd.dma_start(t[:, :], y_t[i])
                tv = t.rearrange("p (r c) -> p r c", c=D)
                hv = head.rearrange("p (r c) -> p r c", c=5)
                nc.vector.tensor_copy(tv[:, :, 0:5], hv[:, :, :])
                nc.gpsimd.dma_start(o_t[i], t[:, :])
    nc.finalize()
    return nc


def _get_nc():
    if "nc" not in _CACHE:
        _CACHE["nc"] = build_nc()
    return _CACHE["nc"]


def make_in_maps(x: np.ndarray, y: np.ndarray):
    h = np.empty((P, 4), np.float32)
    h[:, 0] = x[0, 0]
    h[:, 1] = x[0, 1]
    h[:, 2] = y[0, 0]
    h[:, 3] = y[0, 1]
    return [
        {"y": np.ascontiguousarray(y[i * ROWS_PER_CORE:(i + 1) * ROWS_PER_CORE]), "h": h}
        for i in range(N_CORES)
    ]


def kernel(x: np.ndarray, y: np.ndarray) -> np.ndarray:
    from concourse.bass_utils import run_bass_kernel_spmd

    nc = _get_nc()
    in_maps = make_in_maps(x, y)
    res = run_bass_kernel_spmd(nc, in_maps, core_ids=list(range(N_CORES)))
    return np.concatenate([res.results[i]["o"] for i in range(N_CORES)], axis=0)
